# revision 19
# baseline (speedup 1.0000x reference)
"""ConvSNN forward on 8 Trainium2 NeuronCores.

Math (matches the reference nn.Module):
    post_current = conv2d(x, w, 3x3, stride 1, pad 1)   # [B, 256, 56, 56]
    spikes       = (post_current >= 1.0) ? 1.0 : 0.0

Strategy:
  - Data parallel: 32 images -> 8 cores x 4 images. Weight replicated.
  - The host pre-pads each image into a flat [128, 2+58*57] layout:
    one leading zero guard, then 58 rows (zero row, 56 image rows, zero
    row) of 57 elements (56 cols + 1 zero separator col), one trailing
    guard. A 3x3 tap (kh, kw) is then the [8, 56]-window (row stride 57)
    at flat offset kw + 57*(r0+kh): the separator/guard zeros supply the
    left/right conv padding. Fully contiguous DMAs, no memsets, and
    every matmul is full width (innermost free count 56 is even, which
    the fp32r PE path requires).
  - Conv = 9 accumulating matmuls per PSUM tile (one per tap), weights
    stationary [C_in=128 x 128], moving operand = shifted image window.
  - C_out = 256 -> two 128-row halves (PSUM partition limit).
  - Output pixels tiled 8 rows (448 px) per PSUM bank (<= 512 fp32).
  - Spike threshold: tensor_scalar is_ge 1.0 (PSUM -> SBUF), DMA out.
  - Latency hiding: weights flow on the scalar HWDGE ring in parallel
    with images on the sync ring; each image is loaded as two row-halves
    (chunks 0-3 / 4-6) in separate tiles so compute starts after half an
    image; each PSUM group runs its 9 hi-taps first so only the hi
    operands gate the group start.

Precision: the TRN2 fp32r matmul path runs at full PE rate (1 col/cycle
vs 4 for fp32) but rounds operands to 11 mantissa bits (RNE, measured on
hardware). Modes:
  - "fp32":   exact fp32 matmuls, 4 cyc/row.
  - "fp32r":  single-pass fp32r, 1 cyc/row. Host pre-rounds operands
              (same values the hw cast would produce) so tensors can be
              declared float32r and DMA'd on the fast non-casting path.
  - "fp32r3": hi/lo split-correction, 3 cyc/row, ~fp32 accuracy:
              x = xh + xl, w = wh + wl exactly (RNE-11 splits), and
              conv(x, w) ~= xh*wh + xl*wh + xh*wl (xl*wl ~ 2^-24 dropped).
"""

import numpy as np

B_FULL = 32
N_CORES = 8
B_LOCAL = B_FULL // N_CORES  # 4
C_IN = 128
C_OUT = 256
H = W = 56
KS = 3
WPAD = W + 1        # 56 cols + zero separator
HPAD = H + 2        # zero row, image, zero row
FLAT = 1 + HPAD * WPAD + 1  # leading/trailing zero guards for tap shifts
ROWS_PER_CHUNK = 8
N_CHUNKS = H // ROWS_PER_CHUNK  # 7

# image row-bands: sub s covers chunks 2s..2s+1 (padded rows 16s..16s+17,
# 10 rows for the last band) as flat slices incl. all three column shifts
CHUNKS_PER_SUB = 1
N_SUBS = (N_CHUNKS + CHUNKS_PER_SUB - 1) // CHUNKS_PER_SUB  # 4
SUB_ROW0 = [CHUNKS_PER_SUB * ROWS_PER_CHUNK * s for s in range(N_SUBS)]
SUB_NROWS = [
    min(CHUNKS_PER_SUB * ROWS_PER_CHUNK + 2, HPAD - SUB_ROW0[s])
    for s in range(N_SUBS)
]
SUB_OFF = [WPAD * SUB_ROW0[s] for s in range(N_SUBS)]
SUB_LEN = [SUB_NROWS[s] * WPAD + 2 for s in range(N_SUBS)]

MODE = "fp32r"
# set by test.py to get an NTFF profile (exec_time_ns) out of the run
PROFILE = False
LAST_RESULT = None

_PROG_CACHE = {}


def _round_f32r(v):
    """Round fp32 -> float32r (RNE dropping 12 mantissa bits), bit-exact
    with what the hardware's fp32->fp32r cast produces."""
    u = v.view(np.uint32).astype(np.uint64)
    half = np.uint64(1 << 11)
    lsb = (u >> np.uint64(12)) & np.uint64(1)
    r = ((u + half - np.uint64(1) + lsb) >> np.uint64(12)) << np.uint64(12)
    return r.astype(np.uint32).view(np.float32)


def _build_program(mode):
    import concourse.bacc as bacc
    import concourse.mybir as mybir
    import concourse.tile as tile

    f32 = mybir.dt.float32
    f32r = mybir.dt.float32r
    mm_dt = f32 if mode == "fp32" else f32r
    n_terms = 2 if mode == "fp32r3" else 1  # hi/lo planes per tensor
    # (x_term, w_term) products: hi*hi first so only hi operands gate the
    # group start; then the two first-order corrections for fp32r3
    terms = [(0, 0)] if n_terms == 1 else [(0, 0), (1, 0), (0, 1)]

    nc = bacc.Bacc("TRN2", target_bir_lowering=False, debug=False,
                   num_devices=N_CORES)
    x_d = nc.dram_tensor("x", [B_LOCAL, C_IN, n_terms, FLAT], mm_dt,
                         kind="ExternalInput").ap()
    w_d = nc.dram_tensor("w", [C_IN, n_terms, KS * KS, C_OUT], mm_dt,
                         kind="ExternalInput").ap()
    y_d = nc.dram_tensor("y", [B_LOCAL, C_OUT, H, W], f32,
                         kind="ExternalOutput").ap()

    with tile.TileContext(nc) as tc:
        with (
            tc.tile_pool(name="wpool", bufs=1) as wpool,
            tc.tile_pool(name="xpool", bufs=2) as xpool,
            tc.tile_pool(name="opool", bufs=6) as opool,
            tc.tile_pool(name="psum", bufs=8, space="PSUM") as pspool,
        ):
            # weights in 3-tap groups so the first PSUM group only waits
            # on a 384KB slice
            w_sb = []
            for t in range(n_terms):
                wg = []
                for g in range(KS):
                    wt = wpool.tile([C_IN, KS, C_OUT], mm_dt, tag=f"w{t}{g}")
                    nc.scalar.dma_start(wt[:], w_d[:, t, g * KS:(g + 1) * KS])
                    wg.append(wt)
                w_sb.append(wg)

            for img in range(B_LOCAL):
                subs = []
                for t in range(n_terms):
                    row = []
                    for s in range(N_SUBS):
                        st = xpool.tile([C_IN, SUB_LEN[s]], mm_dt,
                                        tag=f"x{t}{s}")
                        nc.sync.dma_start(
                            st[:],
                            x_d[img, :, t, SUB_OFF[s]:SUB_OFF[s] + SUB_LEN[s]])
                        row.append(st)
                    subs.append(row)
                # views[t][sub][kw]: [rows, WPAD] column-shifted windows
                views = [
                    [
                        [
                            subs[t][s][:, kw:kw + SUB_NROWS[s] * WPAD]
                            .rearrange("p (r c) -> p r c", c=WPAD)
                            for kw in range(KS)
                        ]
                        for s in range(N_SUBS)
                    ]
                    for t in range(n_terms)
                ]

                for half in range(2):
                    # chunks in pairs: consecutive matmuls share the same
                    # stationary weight and alternate PSUM banks
                    groups = [(0, 1), (2, 3), (4, 5), (6,)]
                    for grp in groups:
                        pss = {
                            c: pspool.tile([128, ROWS_PER_CHUNK, W], f32,
                                           tag="ps", name=f"ps_{img}_{half}_{c}")
                            for c in grp
                        }
                        n_mm = KS * KS * len(terms)
                        i = 0
                        for (xtrm, wtrm) in terms:
                            for k in range(KS * KS):
                                kh, kw = divmod(k, KS)
                                lhsT = w_sb[wtrm][k // KS][
                                    :, k % KS, half * 128:(half + 1) * 128]
                                for c in grp:
                                    r0 = c * ROWS_PER_CHUNK
                                    sub = c // CHUNKS_PER_SUB
                                    lr0 = r0 - SUB_ROW0[sub]
                                    rhs = views[xtrm][sub][kw][
                                        :, lr0 + kh:lr0 + kh
                                        + ROWS_PER_CHUNK, :W]
                                    nc.tensor.matmul(pss[c][:], lhsT, rhs,
                                                     start=(i == 0),
                                                     stop=(i == n_mm - 1))
                                i += 1
                        for c in grp:
                            r0 = c * ROWS_PER_CHUNK
                            ot = opool.tile([128, ROWS_PER_CHUNK, W], f32,
                                            tag="o")
                            nc.any.tensor_scalar(ot[:], pss[c][:], 1.0,
                                                 None,
                                                 mybir.AluOpType.is_ge)
                            nc.sync.dma_start(
                                y_d[img, half * 128:(half + 1) * 128,
                                    r0:r0 + ROWS_PER_CHUNK, :],
                                ot[:])
    nc.compile()
    return nc


def _get_program(mode):
    if mode not in _PROG_CACHE:
        _PROG_CACHE[mode] = _build_program(mode)
    return _PROG_CACHE[mode]


def _pad_images(x):
    """[B, C, 56, 56] -> flat guarded layout [B, C, FLAT] (see module doc)."""
    b = x.shape[0]
    xp = np.zeros((b, C_IN, FLAT), dtype=np.float32)
    view = xp[:, :, 1:-1].reshape(b, C_IN, HPAD, WPAD)
    view[:, :, 1:H + 1, :W] = x
    return xp


def kernel(x, weight):
    global LAST_RESULT
    from concourse.bass_utils import run_bass_kernel_spmd

    x = np.asarray(x, dtype=np.float32)
    w = np.asarray(weight, dtype=np.float32)
    # weight[o, c*9 + kh*3 + kw] -> w_sb[c, kh*3+kw, o] (lhsT layout:
    # partition dim = contraction C_in, free dim = C_out)
    w_t = (w.reshape(C_OUT, C_IN, KS, KS).transpose(1, 2, 3, 0)
           .reshape(C_IN, KS * KS, C_OUT))

    if MODE == "fp32":
        x_terms = _pad_images(x)[:, :, None, :]
        w_terms = np.ascontiguousarray(w_t[:, None])
    elif MODE == "fp32r":
        x_terms = _pad_images(_round_f32r(x))[:, :, None, :]
        w_terms = np.ascontiguousarray(_round_f32r(w_t)[:, None])
    else:  # fp32r3: exact hi/lo splits
        xh = _round_f32r(x)
        wh = _round_f32r(w_t)
        xp = _pad_images(xh)
        xlp = _pad_images(_round_f32r(x - xh))
        x_terms = np.ascontiguousarray(np.stack([xp, xlp], axis=2))
        w_terms = np.ascontiguousarray(
            np.stack([wh, _round_f32r(w_t - wh)], axis=1))

    nc = _get_program(MODE)
    in_maps = [
        {"x": x_terms[i * B_LOCAL:(i + 1) * B_LOCAL], "w": w_terms}
        for i in range(N_CORES)
    ]
    res = run_bass_kernel_spmd(nc, in_maps, list(range(N_CORES)),
                               trace=PROFILE)
    LAST_RESULT = res
    return np.concatenate([res.results[i]["y"] for i in range(N_CORES)],
                          axis=0)
